# revision 10
# baseline (speedup 1.0000x reference)
"""GNN mean-aggregation (PyG MessagePassing, aggr='mean') on 8 TRN2 NeuronCores.

Single fused device program per invocation:
  - 2 directions x 2 rounds of mean aggregation, counts, division, and the
    inter-round allgather all execute in ONE program launch.
  - Edges are dst-sharded across cores (core c owns dst in [12500c,12500(c+1)))
    and packed per 128-node dst block into slots of 128 edges, padded to a
    uniform U chunks per block (pad slots carry lo=128 so their one-hot row is
    zero and they contribute nothing, including to counts).
  - x is padded with a 17th all-ones column; the one-hot matmul then
    accumulates counts in the 17th accumulator column for free.
  - Hardware For_i loop over the 98 dst blocks; per iteration: one DMA stages
    the block's 34 offset columns from DRAM, one DVE is_equal builds all 34
    one-hots, then 34 (indirect-gather, matmul) pairs accumulate into PSUM.
  - Division on DVE (round 1 computes 1/max(cnt,1), round 2 reuses it); h1 is
    written to DRAM and allgathered across the 8 cores for round 2's gather.
Host only packs index metadata (static per graph) and reassembles outputs.
"""
import sys
sys.path.insert(0, '/opt/trn_rl_repo')
import numpy as np

import concourse.bass as bass
import concourse.tile as tile
from concourse import bacc, mybir
from concourse.bass import ds
from concourse.bass_utils import run_bass_kernel_spmd

N_NODES = 100000
DIM = 16
DIMP = DIM + 1                   # payload width: 16 dims + count column
DIMB = 32                        # PSUM stride per block (128B; never straddles a 2KB bank)
N_EDGES = 3200000
N_CORES = 8
NLOC = N_NODES // N_CORES        # 12500 dst nodes per core
NBLK = (NLOC + 127) // 128       # 98 blocks per core
RPAD = NBLK * 128                # 12544 padded rows per core
NFULL = N_CORES * RPAD           # 100352 rows in the replicated h layout
PAD_LO = 128.0                   # one-hot sentinel (never matches iota 0..127)

_PROGRAMS = {}


def _balance_blocks(edge_index, reverse_edge_index):
    """Assign each core's nodes to dst blocks, balancing both directions'
    in-degree sums (greedy, descending combined degree). Returns
    rowmap [N_NODES] int64: node -> row (c*128+p)*NBLK + b in the padded,
    partition-major replicated layout."""
    df = np.bincount(np.asarray(edge_index[1]), minlength=N_NODES)
    dr = np.bincount(np.asarray(reverse_edge_index[1]), minlength=N_NODES)
    rowmap = np.empty(N_NODES, np.int64)
    for c in range(N_CORES):
        sl = slice(c * NLOC, (c + 1) * NLOC)
        f, r = df[sl].astype(np.int64), dr[sl].astype(np.int64)
        order = np.argsort(-(f + r), kind='stable')
        lf = np.zeros(NBLK, np.int64)
        lr = np.zeros(NBLK, np.int64)
        cnt = np.zeros(NBLK, np.int64)
        blk = np.empty(NLOC, np.int64)
        lo = np.empty(NLOC, np.int64)
        for i in order:
            cand = np.maximum(lf + f[i], lr + r[i]) + np.where(
                cnt >= 128, 1 << 40, 0)
            b = int(np.argmin(cand))
            blk[i] = b
            lo[i] = cnt[b]
            lf[b] += f[i]
            lr[b] += r[i]
            cnt[b] += 1
        rowmap[sl] = (c * 128 + lo) * NBLK + blk
    return rowmap


def _shard_fixed(edge_index, U, rowmap):
    """Per (core, dst-block) slots padded to U chunks.

    Returns gsrc [8, 128, NS] int32 (rows in the padded h layout) and
    glo [8, 128, NS] f32 (dst lo in 0..127, PAD_LO for padding)."""
    src = np.asarray(edge_index[0], dtype=np.int64)
    dst = np.asarray(edge_index[1], dtype=np.int64)
    drow = rowmap[dst]
    core = dst // NLOC
    rl = drow - core * 128 * NBLK
    lo = rl // NBLK
    key = core * NBLK + rl % NBLK
    order = np.argsort(key, kind='stable')
    ss, ls = src[order], lo[order]
    counts = np.bincount(key, minlength=N_CORES * NBLK).reshape(N_CORES, NBLK)
    assert counts.max() <= U * 128, (counts.max(), U * 128)
    NS = NBLK * U
    rows = rowmap[ss].astype(np.int32)
    gsrc = np.zeros((N_CORES, NS * 128), np.int32)
    glo = np.full((N_CORES, NS * 128), PAD_LO, np.float32)
    starts = np.zeros(N_CORES * NBLK + 1, np.int64)
    np.cumsum(counts.ravel(), out=starts[1:])
    for c in range(N_CORES):
        for b in range(NBLK):
            k = c * NBLK + b
            n = counts[c, b]
            s0 = starts[k]
            base = (b * U) * 128
            gsrc[c, base:base + n] = rows[s0:s0 + n]
            glo[c, base:base + n] = ls[s0:s0 + n]
    # slot-major [NS*128] -> partition-wrapped [128, NS]
    gsrc = np.ascontiguousarray(gsrc.reshape(N_CORES, NS, 128).transpose(0, 2, 1))
    glo = np.ascontiguousarray(glo.reshape(N_CORES, NS, 128).transpose(0, 2, 1))
    return gsrc, glo


def _pack_x(x, rowmap):
    """x [100000,16] -> padded partition-major layout [NFULL, 17]."""
    xp = np.zeros((NFULL, DIMP), np.float32)
    xp[rowmap, :DIM] = x
    xp[rowmap, DIM] = 1.0
    return xp


def _build_fused(U):
    NS = NBLK * U
    nc = bacc.Bacc("TRN2", target_bir_lowering=False, debug=False,
                   num_devices=N_CORES)
    xpad = nc.dram_tensor("xpad", [NFULL, DIMP], mybir.dt.float32,
                          kind="ExternalInput")
    gsrc = [nc.dram_tensor(f"gsrc{d}", [128, NS], mybir.dt.int32,
                           kind="ExternalInput") for d in range(2)]
    glo = [nc.dram_tensor(f"glo{d}", [128, NS], mybir.dt.float32,
                          kind="ExternalInput") for d in range(2)]
    iotaw = nc.dram_tensor("iotaw", [128, 2 * U * 128], mybir.dt.float32,
                           kind="ExternalInput")
    outs = [nc.dram_tensor(f"o{i}", [128, NBLK * DIM], mybir.dt.float32,
                           kind="ExternalOutput") for i in range(4)]
    hself = [nc.dram_tensor(f"hself{d}", [RPAD, DIMP], mybir.dt.float32)
             for d in range(2)]
    hfull = [nc.dram_tensor(f"hfull{d}", [NFULL, DIMP], mybir.dt.float32,
                            addr_space="Shared") for d in range(2)]

    with tile.TileContext(nc) as tc:
        with (
            tc.tile_pool(name="const", bufs=1) as constp,
            tc.tile_pool(name="stage", bufs=2) as stagep,
            tc.tile_pool(name="msg", bufs=2) as msgp,
            tc.tile_pool(name="oh", bufs=2) as ohp,
            tc.tile_pool(name="work", bufs=2) as workp,
            tc.tile_pool(name="psum", bufs=1, space="PSUM") as psump,
        ):
            iota = constp.tile([128, 2 * U, 128], mybir.dt.float32)
            nc.sync.dma_start(out=iota[:], in_=iotaw.ap()[:])
            lo_t = {}
            for d in range(2):
                llt = constp.tile([128, NS, 1], mybir.dt.float32, tag=f"lo{d}")
                nc.sync.dma_start(out=llt[:], in_=glo[d].ap()[:])
                lo_t[d] = llt

            for d in range(2):
                rinv = workp.tile([128, NBLK, 1], mybir.dt.float32,
                                  tag=f"rinv{d}")
                for r in range(2):
                    src = xpad if r == 0 else hfull[d]
                    acc = psump.tile([128, NBLK, DIMB], mybir.dt.float32,
                                     space="PSUM", tag="acc")
                    with tc.For_i(0, NBLK, 2) as b:
                        jU = b * U
                        gstage = stagep.tile([128, 2 * U], mybir.dt.int32,
                                             tag="gs")
                        nc.sync.dma_start(out=gstage[:],
                                          in_=gsrc[d].ap()[:, ds(jU, 2 * U)])
                        oh = ohp.tile([128, 2 * U, 128], mybir.dt.float32,
                                      tag="oh")
                        nc.vector.tensor_tensor(
                            out=oh[:],
                            in0=lo_t[d][:, ds(jU, 2 * U), :].to_broadcast(
                                [128, 2 * U, 128]),
                            in1=iota[:], op=mybir.AluOpType.is_equal)
                        for du in range(2):
                            for u in range(U):
                                k = du * U + u
                                msg = msgp.tile([128, DIMP],
                                                mybir.dt.float32,
                                                tag=f"msg{k % 4}")
                                nc.gpsimd.indirect_dma_start(
                                    out=msg[:], out_offset=None,
                                    in_=src.ap()[:, :],
                                    in_offset=bass.IndirectOffsetOnAxis(
                                        ap=gstage[:, k:k + 1], axis=0))
                                nc.tensor.matmul(
                                    out=acc[:, ds(b + du, 1), 0:DIMP],
                                    lhsT=oh[:, k, :], rhs=msg[:],
                                    start=(u == 0), stop=(u == U - 1))
                    h_sb = workp.tile([128, NBLK, DIMP], mybir.dt.float32,
                                      tag="h")
                    nc.vector.tensor_copy(out=h_sb[:], in_=acc[:, :, 0:DIMP])
                    if r == 0:
                        cnt_sb = workp.tile([128, NBLK, 1], mybir.dt.float32,
                                            tag="cnts")
                        nc.vector.tensor_scalar_max(
                            out=cnt_sb[:], in0=h_sb[:, :, DIM:DIMP],
                            scalar1=1.0)
                        nc.vector.reciprocal(out=rinv[:], in_=cnt_sb[:])
                    nc.vector.tensor_tensor(
                        out=h_sb[:], in0=h_sb[:],
                        in1=rinv[:].to_broadcast([128, NBLK, DIMP]),
                        op=mybir.AluOpType.mult)
                    nc.sync.dma_start(out=outs[2 * d + r].ap()[:],
                                      in_=h_sb[:, :, 0:DIM])
                    if r == 0:
                        nc.sync.dma_start(out=hself[d].ap()[:, :],
                                          in_=h_sb[:])
                        nc.gpsimd.collective_compute(
                            "AllGather", mybir.AluOpType.bypass,
                            replica_groups=[list(range(N_CORES))],
                            ins=[hself[d].ap().opt()],
                            outs=[hfull[d].ap().opt()])
    nc.compile()
    return nc


def _iota_np(U):
    i = np.tile(np.arange(128, dtype=np.float32), (128, 2 * U, 1))
    return np.ascontiguousarray(i.reshape(128, 2 * U * 128))


def _compute_U(edge_index, reverse_edge_index, rowmap):
    U = 0
    for ei in (edge_index, reverse_edge_index):
        dst = np.asarray(ei[1], dtype=np.int64)
        drow = rowmap[dst]
        core = dst // NLOC
        rl = drow - core * 128 * NBLK
        key = core * NBLK + rl % NBLK
        counts = np.bincount(key, minlength=N_CORES * NBLK)
        U = max(U, int((counts.max() + 127) // 128))
    return U


def make_inputs(topic_entity_one_hot, edge_index, reverse_edge_index, U,
                rowmap=None):
    if rowmap is None:
        rowmap = _balance_blocks(edge_index, reverse_edge_index)
    x = np.asarray(topic_entity_one_hot, dtype=np.float32)
    xp = _pack_x(x, rowmap)
    gf, lf = _shard_fixed(np.asarray(edge_index), U, rowmap)
    gr, lr = _shard_fixed(np.asarray(reverse_edge_index), U, rowmap)
    iota = _iota_np(U)
    return [{"xpad": xp, "gsrc0": gf[c], "glo0": lf[c],
             "gsrc1": gr[c], "glo1": lr[c], "iotaw": iota}
            for c in range(N_CORES)]


def kernel(topic_entity_one_hot, edge_index, reverse_edge_index):
    rowmap = _balance_blocks(edge_index, reverse_edge_index)
    U = _compute_U(edge_index, reverse_edge_index, rowmap)
    if ("fused", U) not in _PROGRAMS:
        _PROGRAMS[("fused", U)] = _build_fused(U)
    prog = _PROGRAMS[("fused", U)]
    in_maps = make_inputs(topic_entity_one_hot, edge_index,
                          reverse_edge_index, U, rowmap)
    res = run_bass_kernel_spmd(prog, in_maps, list(range(N_CORES)))
    full = np.concatenate(
        [res.results[c][f"o{i}"].reshape(128 * NBLK, DIM)[None]
         for i in range(4) for c in range(N_CORES)], axis=0)
    full = full.reshape(4, N_CORES * 128 * NBLK, DIM)
    return np.ascontiguousarray(full[:, rowmap, :])


# revision 11
# speedup vs baseline: 1.1067x; 1.1067x over previous
"""GNN mean-aggregation (PyG MessagePassing, aggr='mean') on 8 TRN2 NeuronCores.

Single fused device program per invocation:
  - 2 directions x 2 rounds of mean aggregation, counts, division, and the
    inter-round allgather all execute in ONE program launch.
  - Edges are dst-sharded across cores (core c owns dst in [12500c,12500(c+1))).
    Within a core, nodes are assigned to 98 dst blocks by a greedy balance of
    both directions' in-degrees (minimizes the padded chunk count U); each
    block's edges are packed into slots of 128, padded to U chunks (pad slots
    carry lo=128 so their one-hot row is zero, contributing nothing, counts
    included).
  - x is padded with a 17th all-ones column; the one-hot matmul then
    accumulates counts in the 17th accumulator column for free.
  - Hardware For_i loop over the 98 dst blocks, two blocks per iteration: one
    DMA stages both blocks' 2U offset columns from DRAM, one DVE is_equal
    builds all 2U one-hots at once, then 2U (indirect-gather, matmul) pairs
    accumulate into PSUM (per-block stride padded to 32 floats so no matmul
    output straddles a 2KB PSUM bank).
  - Division on DVE (round 1 computes 1/max(cnt,1), round 2 reuses it); h1 is
    written to DRAM and allgathered across the 8 cores for round 2's gather.
Host only packs index metadata (static per graph) and reassembles outputs.
"""
import sys
sys.path.insert(0, '/opt/trn_rl_repo')
import numpy as np

import concourse.bass as bass
import concourse.tile as tile
from concourse import bacc, mybir
from concourse.bass import ds
from concourse.bass_utils import run_bass_kernel_spmd

N_NODES = 100000
DIM = 16
DIMP = DIM + 1                   # payload width: 16 dims + count column
DIMB = 32                        # PSUM stride per block (128B; never straddles a 2KB bank)
N_EDGES = 3200000
N_CORES = 8
NLOC = N_NODES // N_CORES        # 12500 dst nodes per core
NBLK = (NLOC + 127) // 128       # 98 blocks per core
RPAD = NBLK * 128                # 12544 padded rows per core
NFULL = N_CORES * RPAD           # 100352 rows in the replicated h layout
PAD_LO = 128.0                   # one-hot sentinel (never matches iota 0..127)

_PROGRAMS = {}


def _balance_blocks(edge_index, reverse_edge_index):
    """Assign each core's nodes to dst blocks, balancing both directions'
    in-degree sums (greedy, descending combined degree). Returns
    rowmap [N_NODES] int64: node -> row (c*128+p)*NBLK + b in the padded,
    partition-major replicated layout."""
    df = np.bincount(np.asarray(edge_index[1]), minlength=N_NODES)
    dr = np.bincount(np.asarray(reverse_edge_index[1]), minlength=N_NODES)
    rowmap = np.empty(N_NODES, np.int64)
    for c in range(N_CORES):
        sl = slice(c * NLOC, (c + 1) * NLOC)
        f, r = df[sl].astype(np.int64), dr[sl].astype(np.int64)
        order = np.argsort(-(f + r), kind='stable')
        lf = np.zeros(NBLK, np.int64)
        lr = np.zeros(NBLK, np.int64)
        cnt = np.zeros(NBLK, np.int64)
        blk = np.empty(NLOC, np.int64)
        lo = np.empty(NLOC, np.int64)
        for i in order:
            cand = np.maximum(lf + f[i], lr + r[i]) + np.where(
                cnt >= 128, 1 << 40, 0)
            b = int(np.argmin(cand))
            blk[i] = b
            lo[i] = cnt[b]
            lf[b] += f[i]
            lr[b] += r[i]
            cnt[b] += 1
        rowmap[sl] = (c * 128 + lo) * NBLK + blk
    return rowmap


def _shard_fixed(edge_index, U, rowmap):
    """Per (core, dst-block) slots padded to U chunks.

    Returns gsrc [8, 128, NS] int32 (rows in the padded h layout) and
    glo [8, 128, NS] f32 (dst lo in 0..127, PAD_LO for padding)."""
    src = np.asarray(edge_index[0], dtype=np.int64)
    dst = np.asarray(edge_index[1], dtype=np.int64)
    drow = rowmap[dst]
    core = dst // NLOC
    rl = drow - core * 128 * NBLK
    lo = rl // NBLK
    key = core * NBLK + rl % NBLK
    order = np.argsort(key, kind='stable')
    ss, ls = src[order], lo[order]
    counts = np.bincount(key, minlength=N_CORES * NBLK).reshape(N_CORES, NBLK)
    assert counts.max() <= U * 128, (counts.max(), U * 128)
    NS = NBLK * U
    rows = rowmap[ss].astype(np.int32)
    gsrc = np.zeros((N_CORES, NS * 128), np.int32)
    glo = np.full((N_CORES, NS * 128), PAD_LO, np.float32)
    starts = np.zeros(N_CORES * NBLK + 1, np.int64)
    np.cumsum(counts.ravel(), out=starts[1:])
    for c in range(N_CORES):
        for b in range(NBLK):
            k = c * NBLK + b
            n = counts[c, b]
            s0 = starts[k]
            base = (b * U) * 128
            gsrc[c, base:base + n] = rows[s0:s0 + n]
            glo[c, base:base + n] = ls[s0:s0 + n]
    # slot-major [NS*128] -> partition-wrapped [128, NS]
    gsrc = np.ascontiguousarray(gsrc.reshape(N_CORES, NS, 128).transpose(0, 2, 1))
    glo = np.ascontiguousarray(glo.reshape(N_CORES, NS, 128).transpose(0, 2, 1))
    return gsrc, glo


def _pack_x(x, rowmap):
    """x [100000,16] -> padded partition-major layout [NFULL, 17]."""
    xp = np.zeros((NFULL, DIMP), np.float32)
    xp[rowmap, :DIM] = x
    xp[rowmap, DIM] = 1.0
    return xp


def _build_fused(U):
    NS = NBLK * U
    nc = bacc.Bacc("TRN2", target_bir_lowering=False, debug=False,
                   num_devices=N_CORES)
    xpad = nc.dram_tensor("xpad", [NFULL, DIMP], mybir.dt.float32,
                          kind="ExternalInput")
    gsrc = [nc.dram_tensor(f"gsrc{d}", [128, NS], mybir.dt.int32,
                           kind="ExternalInput") for d in range(2)]
    glo = [nc.dram_tensor(f"glo{d}", [128, NS], mybir.dt.float32,
                          kind="ExternalInput") for d in range(2)]
    iotaw = nc.dram_tensor("iotaw", [128, 2 * U * 128], mybir.dt.float32,
                           kind="ExternalInput")
    outs = [nc.dram_tensor(f"o{i}", [128, NBLK * DIM], mybir.dt.float32,
                           kind="ExternalOutput") for i in range(4)]
    hself = [nc.dram_tensor(f"hself{d}", [RPAD, DIMP], mybir.dt.float32)
             for d in range(2)]
    hfull = [nc.dram_tensor(f"hfull{d}", [NFULL, DIMP], mybir.dt.float32,
                            addr_space="Shared") for d in range(2)]

    with tile.TileContext(nc) as tc:
        with (
            tc.tile_pool(name="const", bufs=1) as constp,
            tc.tile_pool(name="stage", bufs=2) as stagep,
            tc.tile_pool(name="msg", bufs=2) as msgp,
            tc.tile_pool(name="oh", bufs=2) as ohp,
            tc.tile_pool(name="work", bufs=2) as workp,
            tc.tile_pool(name="psum", bufs=1, space="PSUM") as psump,
        ):
            iota = constp.tile([128, 2 * U, 128], mybir.dt.float32)
            nc.sync.dma_start(out=iota[:], in_=iotaw.ap()[:])
            lo_t = {}
            for d in range(2):
                llt = constp.tile([128, NS, 1], mybir.dt.float32, tag=f"lo{d}")
                nc.sync.dma_start(out=llt[:], in_=glo[d].ap()[:])
                lo_t[d] = llt

            for d in range(2):
                rinv = workp.tile([128, NBLK, 1], mybir.dt.float32,
                                  tag=f"rinv{d}")
                for r in range(2):
                    src = xpad if r == 0 else hfull[d]
                    acc = psump.tile([128, NBLK, DIMB], mybir.dt.float32,
                                     space="PSUM", tag="acc")
                    with tc.For_i(0, NBLK, 2) as b:
                        jU = b * U
                        gstage = stagep.tile([128, 2 * U], mybir.dt.int32,
                                             tag="gs")
                        nc.sync.dma_start(out=gstage[:],
                                          in_=gsrc[d].ap()[:, ds(jU, 2 * U)])
                        oh = ohp.tile([128, 2 * U, 128], mybir.dt.float32,
                                      tag="oh")
                        nc.vector.tensor_tensor(
                            out=oh[:],
                            in0=lo_t[d][:, ds(jU, 2 * U), :].to_broadcast(
                                [128, 2 * U, 128]),
                            in1=iota[:], op=mybir.AluOpType.is_equal)
                        for du in range(2):
                            for u in range(U):
                                k = du * U + u
                                msg = msgp.tile([128, DIMP],
                                                mybir.dt.float32,
                                                tag=f"msg{k % 4}")
                                nc.gpsimd.indirect_dma_start(
                                    out=msg[:], out_offset=None,
                                    in_=src.ap()[:, :],
                                    in_offset=bass.IndirectOffsetOnAxis(
                                        ap=gstage[:, k:k + 1], axis=0))
                                nc.tensor.matmul(
                                    out=acc[:, ds(b + du, 1), 0:DIMP],
                                    lhsT=oh[:, k, :], rhs=msg[:],
                                    start=(u == 0), stop=(u == U - 1))
                    h_sb = workp.tile([128, NBLK, DIMP], mybir.dt.float32,
                                      tag="h")
                    nc.vector.tensor_copy(out=h_sb[:], in_=acc[:, :, 0:DIMP])
                    if r == 0:
                        cnt_sb = workp.tile([128, NBLK, 1], mybir.dt.float32,
                                            tag="cnts")
                        nc.vector.tensor_scalar_max(
                            out=cnt_sb[:], in0=h_sb[:, :, DIM:DIMP],
                            scalar1=1.0)
                        nc.vector.reciprocal(out=rinv[:], in_=cnt_sb[:])
                    nc.vector.tensor_tensor(
                        out=h_sb[:], in0=h_sb[:],
                        in1=rinv[:].to_broadcast([128, NBLK, DIMP]),
                        op=mybir.AluOpType.mult)
                    nc.sync.dma_start(out=outs[2 * d + r].ap()[:],
                                      in_=h_sb[:, :, 0:DIM])
                    if r == 0:
                        nc.sync.dma_start(out=hself[d].ap()[:, :],
                                          in_=h_sb[:])
                        nc.gpsimd.collective_compute(
                            "AllGather", mybir.AluOpType.bypass,
                            replica_groups=[list(range(N_CORES))],
                            ins=[hself[d].ap().opt()],
                            outs=[hfull[d].ap().opt()])
    nc.compile()
    return nc


def _iota_np(U):
    i = np.tile(np.arange(128, dtype=np.float32), (128, 2 * U, 1))
    return np.ascontiguousarray(i.reshape(128, 2 * U * 128))


def _compute_U(edge_index, reverse_edge_index, rowmap):
    U = 0
    for ei in (edge_index, reverse_edge_index):
        dst = np.asarray(ei[1], dtype=np.int64)
        drow = rowmap[dst]
        core = dst // NLOC
        rl = drow - core * 128 * NBLK
        key = core * NBLK + rl % NBLK
        counts = np.bincount(key, minlength=N_CORES * NBLK)
        U = max(U, int((counts.max() + 127) // 128))
    return U


def make_inputs(topic_entity_one_hot, edge_index, reverse_edge_index, U,
                rowmap=None):
    if rowmap is None:
        rowmap = _balance_blocks(edge_index, reverse_edge_index)
    x = np.asarray(topic_entity_one_hot, dtype=np.float32)
    xp = _pack_x(x, rowmap)
    gf, lf = _shard_fixed(np.asarray(edge_index), U, rowmap)
    gr, lr = _shard_fixed(np.asarray(reverse_edge_index), U, rowmap)
    iota = _iota_np(U)
    return [{"xpad": xp, "gsrc0": gf[c], "glo0": lf[c],
             "gsrc1": gr[c], "glo1": lr[c], "iotaw": iota}
            for c in range(N_CORES)]


def kernel(topic_entity_one_hot, edge_index, reverse_edge_index):
    rowmap = _balance_blocks(edge_index, reverse_edge_index)
    U = _compute_U(edge_index, reverse_edge_index, rowmap)
    if ("fused", U) not in _PROGRAMS:
        _PROGRAMS[("fused", U)] = _build_fused(U)
    prog = _PROGRAMS[("fused", U)]
    in_maps = make_inputs(topic_entity_one_hot, edge_index,
                          reverse_edge_index, U, rowmap)
    res = run_bass_kernel_spmd(prog, in_maps, list(range(N_CORES)))
    full = np.concatenate(
        [res.results[c][f"o{i}"].reshape(128 * NBLK, DIM)[None]
         for i in range(4) for c in range(N_CORES)], axis=0)
    full = full.reshape(4, N_CORES * 128 * NBLK, DIM)
    return np.ascontiguousarray(full[:, rowmap, :])
